# revision 39
# baseline (speedup 1.0000x reference)
import sys

sys.path.insert(0, "/opt/trn_rl_repo")

import numpy as np

import jax
import jax.numpy as jnp
from jax.sharding import Mesh, NamedSharding, PartitionSpec
from jax.experimental.shard_map import shard_map

import concourse.bass as bass
import concourse.tile as tile
from concourse import mybir
from concourse.tile import add_dep_helper
from concourse.bass2jax import (
    _bass_exec_p, install_neuronx_cc_hook, partition_id_tensor)

# Problem constants (nn_MoEBlock: B,C,T,H,W = 2,128,8,64,64; E=8; top-2)
B, C, T, H, W = 2, 128, 8, 64, 64
E = 8
THW = T * H * W               # 32768
NVOX = B * THW                # 65536 voxels
NCORES = 8
NSH = NVOX // NCORES          # 8192 voxels per core
CPB = NCORES // B             # cores per batch element (4)
NSPLIT = 4                    # pipeline depth: device exec and both wire
                              # directions overlap across splits (the
                              # per-device queue is FIFO in issue order,
                              # but up- and down-transfers are full duplex)
NS = NSH // NSPLIT            # voxels per core per call
NC_CHUNK = 1024               # main-loop chunk (voxels)
NCHUNKS = NS // NC_CHUNK
F16 = mybir.dt.float16
F32 = mybir.dt.float32
F8 = mybir.dt.float8e4
NP_F8 = mybir.dt.np(F8)       # ml_dtypes.float8_e4m3

# ml_dtypes' elementwise f8 casts go through a slow scalar ufunc path on
# this 1-CPU host; table lookups are ~2x faster for both directions.
with np.errstate(invalid="ignore", over="ignore"):
    # f32 -> f16 (SIMD) -> f8 via 64K-entry table (double rounding is
    # harmless at our 13x error margin)
    _LUT_F16_TO_F8 = (np.arange(65536, dtype=np.uint16).view(np.float16)
                      .astype(NP_F8))
    # f8 -> f32 via 256-entry table
    _LUT_F8_TO_F32 = (np.arange(256, dtype=np.uint8).view(NP_F8)
                      .astype(np.float32))


def _to_f8(a32):
    """fast f32 ndarray -> f8e4m3 ndarray (any shape/strides)"""
    return _LUT_F16_TO_F8[a32.astype(np.float16).view(np.uint16)]


_AR_N2 = np.arange(NCORES * NS)   # voxel index vector reused per split


def _split_waits(nc, max_waits=1):
    """The walrus scheduler accepts only one sync-wait per instruction.
    Move extra on_wait conditions onto standalone same-engine NoOps
    inserted immediately before the instruction (same engine stream =>
    identical semantics)."""
    ctr = 0
    for f in nc.m.functions:
        for bb in f.blocks:
            insts = list(bb.instructions)
            out = []
            changed = False
            for inst in insts:
                si = inst.sync_info
                w = list(si.on_wait) if si is not None and si.on_wait else []
                if (len(w) > max_waits
                        and inst.engine != mybir.EngineType.Unassigned):
                    for extra in w[:-max_waits]:
                        ctr += 1
                        nop = mybir.InstNoOp(
                            name=f"WSPLIT-{ctr}", ins=[], outs=[])
                        nop.engine = inst.engine
                        nop.sync_info = mybir.SyncInfo(
                            on_wait=[extra], on_update=[])
                        out.append(nop)
                    inst.sync_info = mybir.SyncInfo(
                        on_wait=w[-max_waits:],
                        on_update=list(si.on_update) if si.on_update else [])
                    changed = True
                out.append(inst)
            if changed:
                try:
                    bb.instructions = out
                except Exception:
                    bb.instructions.clear()
                    bb.instructions.extend(out)
    return nc


def build_kernel(hasb1: bool, hasb2: bool):
    """Expert MLP only — gating (logits/top-2/softmax) happens on host in
    exact f32; the per-voxel expert weights arrive as the f8 `wc` input.
    Output is the MoE delta (no +x residual; host adds it in f32)."""
    act_fn = mybir.ActivationFunctionType.Silu
    wt_cols = 2 * E * C + (E if hasb1 else 0)
    nc = bass.Bass()
    x8_d = nc.dram_tensor("x8", [C, NS], F8, kind="ExternalInput")
    wc_d = nc.dram_tensor("wc", [E, NS], F8, kind="ExternalInput")
    wt_d = nc.dram_tensor("wt", [C, wt_cols], F16, kind="ExternalInput")
    b2_d = (nc.dram_tensor("b2m", [E, C], F16, kind="ExternalInput")
            if hasb2 else None)
    out_d = nc.dram_tensor("out", [C, NS], F8, kind="ExternalOutput")

    with tile.TileContext(nc) as tc:
        with (
            tc.tile_pool(name="consts", bufs=1) as consts,
            tc.tile_pool(name="xp", bufs=1) as xp,
            tc.tile_pool(name="fpool", bufs=3) as fpool,
            tc.tile_pool(name="gpool", bufs=3) as gpool,
            tc.tile_pool(name="opool", bufs=2) as opool,
        ):
            # ---------- loads ----------
            x_sb = xp.tile([C, NS], F8)
            wcm_sb = consts.tile([E, NS], F8)
            w1T = consts.tile([C, E * C], F16)
            w2T = consts.tile([C, E * C], F16)
            sel32 = consts.tile([E, E * C], F32)
            sel = consts.tile([E, E * C], F16)

            dmas = []
            for j in range(4):
                s = slice(j * (NS // 4), (j + 1) * (NS // 4))
                dmas.append(nc.sync.dma_start(x_sb[:, s], x8_d[:, s]))
            dmas.append(nc.sync.dma_start(wcm_sb[:], wc_d[:]))
            dmas.append(nc.sync.dma_start(w1T[:], wt_d[:, 0:E * C]))
            dmas.append(nc.sync.dma_start(w2T[:], wt_d[:, E * C:2 * E * C]))
            if hasb1:
                b1m = consts.tile([C, E], F32)
                bh = consts.tile([C, E], F16)
                dmas.append(nc.sync.dma_start(
                    bh[:], wt_d[:, 2 * E * C:2 * E * C + E]))
                nc.scalar.copy(b1m[:], bh[:])
            if hasb2:
                b2m = consts.tile([E, C], F16)
                dmas.append(nc.sync.dma_start(b2m[:], b2_d[:]))

            # on-device constant: expert-broadcast selector
            # sel[e, e*C:(e+1)*C] = 1, else 0
            nc.vector.memset(sel32[:], 1.0)
            nc.gpsimd.affine_select(
                sel32[:], sel32[:], pattern=[[1, E * C]], base=0,
                channel_multiplier=-C,
                compare_op=mybir.AluOpType.is_ge, fill=0.0)
            nc.gpsimd.affine_select(
                sel32[:], sel32[:], pattern=[[-1, E * C]], base=C - 1,
                channel_multiplier=C,
                compare_op=mybir.AluOpType.is_ge, fill=0.0)
            sel_cp = nc.scalar.copy(sel[:], sel32[:])

            # PE carries only ONE sync wait per Matmult through walrus;
            # absorb each input-DMA dependency into a PE nop up front.
            dma_nops = []
            for dma in dmas:
                nop = nc.tensor.nop(nofuse=True)
                add_dep_helper(nop.ins, dma.ins, sync=True)
                dma_nops.append(nop)

            def pe_absorb(producers, consumer_mms):
                nops = []
                for p in producers:
                    if p is None:
                        continue
                    n = nc.tensor.nop(nofuse=True)
                    add_dep_helper(n.ins, p.ins, sync=True)
                    nops.append(n)
                for m in consumer_mms:
                    for n in nops:
                        add_dep_helper(m.ins, n.ins, sync=False)

            # ---------- experts + combine ----------
            mpsum = tc.tile_pool(name="ps_m", bufs=1, space="PSUM")
            ps_m = mpsum.__enter__()
            mpsum2 = tc.tile_pool(name="ps_m2", bufs=2, space="PSUM")
            ps_m2 = mpsum2.__enter__()
            prev_out = prev_l2last = None
            hist_silu = [None, None]
            hist_mult = [None, None]
            hist_hmm = [None, None]
            hist_wb = [None, None]
            for i in range(NCHUNKS):
                cs = slice(i * NC_CHUNK, (i + 1) * NC_CHUNK)
                pso = ps_m.tile([C, NC_CHUNK], F32, tag="pso")
                for e in range(E):
                    mms = []
                    psh = ps_m2.tile([C, NC_CHUNK], F32, tag="psh")
                    for s in range(NC_CHUNK // 512):
                        rs = slice(i * NC_CHUNK + s * 512,
                                   i * NC_CHUNK + (s + 1) * 512)
                        mms.append(nc.tensor.matmul(
                            psh[:, s * 512:(s + 1) * 512],
                            w1T[:, e * C:(e + 1) * C],
                            x_sb[:, rs],
                            start=True, stop=True))
                    f_sb = fpool.tile([C, NC_CHUNK], F16, tag="f")
                    if hasb1:
                        silu_ins = nc.scalar.activation(
                            f_sb[:], psh[:], act_fn, bias=b1m[:, e:e + 1])
                    else:
                        silu_ins = nc.scalar.activation(f_sb[:], psh[:], act_fn)
                    pswb = ps_m.tile([C, NC_CHUNK], F32, tag="pswb")
                    for s in range(NC_CHUNK // 512):
                        ws = slice(i * NC_CHUNK + s * 512,
                                   i * NC_CHUNK + (s + 1) * 512)
                        mms.append(nc.tensor.matmul(
                            pswb[:, s * 512:(s + 1) * 512],
                            sel[:, e * C:(e + 1) * C], wcm_sb[:, ws],
                            start=True, stop=True))
                    g_sb = gpool.tile([C, NC_CHUNK], F16, tag="g")
                    mult_ins = nc.vector.tensor_mul(g_sb[:], f_sb[:], pswb[:])
                    for s in range(NC_CHUNK // 512):
                        ss = slice(s * 512, (s + 1) * 512)
                        mms.append(nc.tensor.matmul(
                            pso[:, ss],
                            w2T[:, e * C:(e + 1) * C],
                            g_sb[:, ss],
                            start=(e == 0),
                            stop=(e == E - 1) and not hasb2))
                    # absorb all cross-engine + psum-WAW deps into PE nops
                    pe_absorb([hist_silu[0], hist_mult[-1], prev_out,
                               hist_hmm[0], hist_wb[-1], prev_l2last,
                               sel_cp if (i == 0 and e == 0) else None],
                              mms[:1])
                    for m in mms[1:]:
                        add_dep_helper(m.ins, mms[0].ins, sync=False)
                    pe_absorb([silu_ins, mult_ins], mms[-NC_CHUNK // 512:])
                    hist_silu = [hist_silu[-1], silu_ins]
                    hist_mult = [hist_mult[-1], mult_ins]
                    hist_hmm = [hist_hmm[-1], mms[1]]
                    hist_wb = [hist_wb[-1], mms[NC_CHUNK // 512 + 1]]
                    if e == E - 1:
                        prev_l2last = mms[-1]
                if hasb2:
                    for s in range(NC_CHUNK // 512):
                        ss = slice(s * 512, (s + 1) * 512)
                        rs = slice(i * NC_CHUNK + s * 512,
                                   i * NC_CHUNK + (s + 1) * 512)
                        nc.tensor.matmul(
                            pso[:, ss], b2m[:], wcm_sb[:, rs],
                            start=False, stop=True)
                # delta only — the +x residual happens on host in f32
                o_sb = opool.tile([C, NC_CHUNK], F8, tag="o")
                prev_out = nc.scalar.copy(o_sb[:], pso[:])
                nc.sync.dma_start(out_d[:, cs], o_sb[:])
            mpsum2.__exit__(None, None, None)
            mpsum.__exit__(None, None, None)
    _split_waits(nc)
    return nc


# --- host side -------------------------------------------------------------
# Replicates bass_utils.run_bass_kernel_spmd's axon execution path
# (shard_map over jax.devices()[:8] + the _bass_exec_p custom call), but
# builds the jitted callable ONCE and reuses it: re-tracing and re-lowering
# on every call costs seconds of host time while computing nothing new.
_state: dict = {}


def _get_state(key):
    if key in _state:
        return _state[key]
    install_neuronx_cc_hook()
    nc = build_kernel(*key)
    partition_name = (nc.partition_id_tensor.name
                      if nc.partition_id_tensor else None)
    in_names, out_names, out_avals = [], [], []
    for alloc in nc.m.functions[0].allocations:
        if not isinstance(alloc, mybir.MemoryLocationSet):
            continue
        name = alloc.memorylocations[0].name
        if alloc.kind == "ExternalInput":
            if name != partition_name:
                in_names.append(name)
        elif alloc.kind == "ExternalOutput":
            out_names.append(name)
            out_avals.append(jax.core.ShapedArray(
                tuple(alloc.tensor_shape), mybir.dt.np(alloc.dtype)))
    n_params = len(in_names)
    n_outs = len(out_avals)
    all_names = tuple(in_names + out_names
                      + ([partition_name] if partition_name else []))

    def _body(*args):
        operands = list(args)
        if partition_name is not None:
            operands.append(partition_id_tensor())
        outs = _bass_exec_p.bind(
            *operands,
            out_avals=tuple(out_avals),
            in_names=all_names,
            out_names=tuple(out_names),
            lowering_input_output_aliases=(),
            sim_require_finite=True,
            sim_require_nnan=True,
            nc=nc,
        )
        return tuple(outs)

    devices = jax.devices()[:NCORES]
    mesh = Mesh(np.asarray(devices), ("core",))
    sharding = NamedSharding(mesh, PartitionSpec("core"))
    donate = tuple(range(n_params, n_params + n_outs))
    njit = jax.jit(
        shard_map(_body, mesh=mesh,
                  in_specs=(PartitionSpec("core"),) * (n_params + n_outs),
                  out_specs=(PartitionSpec("core"),) * n_outs,
                  check_rep=False),
        donate_argnums=donate, keep_unused=True)
    mkdonor = jax.jit(
        lambda: jnp.zeros((NCORES * C, NS), NP_F8), out_shardings=sharding)
    st = {"nc": nc, "njit": njit, "mkdonor": mkdonor, "sharding": sharding,
          "devices": devices, "in_names": in_names, "donors": [],
          "wt_sig": None, "wt_dev": None, "compiled": None}
    _state[key] = st
    return st


def kernel(x, gate_w, gate_b, w1, b1, w2, b2):
    x = np.asarray(x, dtype=np.float32)
    gate_w = np.asarray(gate_w, dtype=np.float32)
    gate_b = np.asarray(gate_b, dtype=np.float32)
    w1 = np.asarray(w1, dtype=np.float32)
    b1 = np.asarray(b1, dtype=np.float32)
    w2 = np.asarray(w2, dtype=np.float32)
    b2 = np.asarray(b2, dtype=np.float32)

    key = (bool(b1.any()), bool(b2.any()))
    st = _get_state(key)

    xr = x.reshape(B, C, THW)

    # x split shards, quantized to fp8, shipped as one async device_put
    # (a single put returns ~40ms sooner than 8 per-device puts on this
    # 1-CPU host, and the upload wire time is identical). The transfer is
    # in flight during the host gating compute below.
    x5 = x.reshape(B, C, CPB, NSPLIT, NS)

    def put_x(sp):
        # one strided gather+quantize pass: [B,CPB,C,NS] row-blocks land
        # exactly at core c = bb*CPB + q
        xg = _to_f8(x5[:, :, :, sp, :].transpose(0, 2, 1, 3)
                    ).reshape(NCORES * C, NS)
        return jax.device_put(xg, st["sharding"])

    x8_dev0 = put_x(0)   # upload in flight during the host work below

    # weights: device-resident across calls; re-uploaded only on change
    cached = st["wt_sig"]
    if (cached is None or not np.array_equal(cached[0], w1)
            or not np.array_equal(cached[1], w2)
            or not np.array_equal(cached[2], b1)):
        wt_cols = 2 * E * C + (E if key[0] else 0)
        wpk = np.empty((C, wt_cols), dtype=np.float16)
        wpk[:, 0:E * C] = w1.T
        wpk[:, E * C:2 * E * C] = w2.transpose(2, 0, 1).reshape(C, E * C)
        if key[0]:
            wpk[:, 2 * E * C:] = b1.reshape(E, C).T
        st["wt_dev"] = jax.device_put(np.tile(wpk, (NCORES, 1)),
                                      st["sharding"])
        st["wt_sig"] = (w1.copy(), w2.copy(), b1.copy())

    # --- gating on host, exact f32, one split at a time ---
    xr4 = x.reshape(B, C, CPB, NSH)
    has_gb = bool(gate_b.any())

    def gate_split(sp):
        lo = sp * NS
        # per-core strided-view gemms: 5x faster than tensordot (which
        # copies the slice before BLAS)
        lg = np.empty((E, NCORES, NS), dtype=np.float32)
        for c in range(NCORES):
            bb, q = divmod(c, CPB)
            o = q * NSH + lo
            np.matmul(gate_w, xr[bb, :, o:o + NS], out=lg[:, c, :])
        if has_gb:
            lg += gate_b[:, None, None]
        lt = lg.reshape(E, -1)
        n2 = lt.shape[1]
        ar = _AR_N2
        i1 = np.argmax(lt, axis=0)
        v1 = lt[i1, ar]
        lt[i1, ar] = -np.inf
        i2 = np.argmax(lt, axis=0)
        v2 = lt[i2, ar]
        ez = 1.0 / (1.0 + np.exp(v2 - v1))             # top-2 softmax
        w32 = np.zeros((E, n2), dtype=np.float32)
        w32[i1, ar] = ez
        w32[i2, ar] = 1.0 - ez
        return _to_f8(
            w32.reshape(E, NCORES, NS).transpose(1, 0, 2)
        ).reshape(NCORES * E, NS)

    b2t = (np.tile(b2.astype(np.float16), (NCORES, 1)) if key[1] else None)
    donors = st["donors"]
    st["donors"] = []

    def dispatch(sp, x8_dev):
        args_np = {"x8": x8_dev, "wc": gate_split(sp), "wt": st["wt_dev"],
                   "b2m": b2t}
        args = [args_np[n] for n in st["in_names"]]
        donor = donors.pop() if donors else st["mkdonor"]()
        fn = st["compiled"] or st["njit"]
        return fn(*args, donor)[0]

    # Issue order is the device queue order: dispatch split sp and kick
    # its result-fetch asyncs (pre-queued, so the data streams back the
    # moment exec sp finishes) BEFORE uploading split sp+1 — the fetch of
    # sp overlaps the upload of sp+1 on the full-duplex link.
    outs = [None] * NSPLIT
    for sp in range(NSPLIT):
        outs[sp] = dispatch(sp, x8_dev0 if sp == 0 else put_x(sp))
        for s in outs[sp].addressable_shards:
            try:
                s.data.copy_to_host_async()
            except Exception:
                pass
    # drain: per split, one gather of the (already async-copied) shards
    # and one 256-entry-LUT pass f8 -> f32, then 8 strided residual adds.
    # Batching per split halves the python/dispatch overhead of 32 tiny
    # per-shard conversions.
    out = np.empty((B, C, THW), dtype=np.float32)
    for sp in range(NSPLIT):
        d32 = _LUT_F8_TO_F32[np.asarray(outs[sp]).view(np.uint8)]
        for c in range(NCORES):
            bb, q = divmod(c, CPB)
            lo = q * NSH + sp * NS
            np.add(xr[bb, :, lo:lo + NS], d32[c * C:(c + 1) * C],
                   out=out[bb, :, lo:lo + NS])
        st["donors"].append(outs[sp])   # recycle as future donations
    return out.reshape(B, C, T, H, W)


# revision 41
# speedup vs baseline: 1.0956x; 1.0956x over previous
import sys

sys.path.insert(0, "/opt/trn_rl_repo")

import numpy as np

import jax
import jax.numpy as jnp
from jax.sharding import Mesh, NamedSharding, PartitionSpec
from jax.experimental.shard_map import shard_map

import concourse.bass as bass
import concourse.tile as tile
from concourse import mybir
from concourse.tile import add_dep_helper
from concourse.bass2jax import (
    _bass_exec_p, install_neuronx_cc_hook, partition_id_tensor)

# Problem constants (nn_MoEBlock: B,C,T,H,W = 2,128,8,64,64; E=8; top-2)
B, C, T, H, W = 2, 128, 8, 64, 64
E = 8
THW = T * H * W               # 32768
NVOX = B * THW                # 65536 voxels
NCORES = 8
NSH = NVOX // NCORES          # 8192 voxels per core
CPB = NCORES // B             # cores per batch element (4)
NSPLIT = 4                    # pipeline depth: device exec and both wire
                              # directions overlap across splits (the
                              # per-device queue is FIFO in issue order,
                              # but up- and down-transfers are full duplex)
NS = NSH // NSPLIT            # voxels per core per call
NC_CHUNK = 1024               # main-loop chunk (voxels)
NCHUNKS = NS // NC_CHUNK
F16 = mybir.dt.float16
F32 = mybir.dt.float32
F8 = mybir.dt.float8e4
NP_F8 = mybir.dt.np(F8)       # ml_dtypes.float8_e4m3

# ml_dtypes' elementwise f8 casts go through a slow scalar ufunc path on
# this 1-CPU host; table lookups are ~2x faster for both directions.
with np.errstate(invalid="ignore", over="ignore"):
    # f32 -> f16 (SIMD) -> f8 via 64K-entry table (double rounding is
    # harmless at our 13x error margin)
    _LUT_F16_TO_F8 = (np.arange(65536, dtype=np.uint16).view(np.float16)
                      .astype(NP_F8))
    # f8 -> f32 via 256-entry table
    _LUT_F8_TO_F32 = (np.arange(256, dtype=np.uint8).view(NP_F8)
                      .astype(np.float32))


def _to_f8(a32):
    """fast f32 ndarray -> f8e4m3 ndarray (any shape/strides)"""
    return _LUT_F16_TO_F8[a32.astype(np.float16).view(np.uint16)]


_AR_N2 = np.arange(NCORES * NS)   # voxel index vector reused per split


def _split_waits(nc, max_waits=1):
    """The walrus scheduler accepts only one sync-wait per instruction.
    Move extra on_wait conditions onto standalone same-engine NoOps
    inserted immediately before the instruction (same engine stream =>
    identical semantics)."""
    ctr = 0
    for f in nc.m.functions:
        for bb in f.blocks:
            insts = list(bb.instructions)
            out = []
            changed = False
            for inst in insts:
                si = inst.sync_info
                w = list(si.on_wait) if si is not None and si.on_wait else []
                if (len(w) > max_waits
                        and inst.engine != mybir.EngineType.Unassigned):
                    for extra in w[:-max_waits]:
                        ctr += 1
                        nop = mybir.InstNoOp(
                            name=f"WSPLIT-{ctr}", ins=[], outs=[])
                        nop.engine = inst.engine
                        nop.sync_info = mybir.SyncInfo(
                            on_wait=[extra], on_update=[])
                        out.append(nop)
                    inst.sync_info = mybir.SyncInfo(
                        on_wait=w[-max_waits:],
                        on_update=list(si.on_update) if si.on_update else [])
                    changed = True
                out.append(inst)
            if changed:
                try:
                    bb.instructions = out
                except Exception:
                    bb.instructions.clear()
                    bb.instructions.extend(out)
    return nc


def build_kernel(hasb1: bool, hasb2: bool):
    """Expert MLP only — gating (logits/top-2/softmax) happens on host in
    exact f32; the per-voxel expert weights arrive as the f8 `wc` input.
    Output is the MoE delta (no +x residual; host adds it in f32)."""
    act_fn = mybir.ActivationFunctionType.Silu
    wt_cols = 2 * E * C + (E if hasb1 else 0)
    nc = bass.Bass()
    x8_d = nc.dram_tensor("x8", [C, NS], F8, kind="ExternalInput")
    wc_d = nc.dram_tensor("wc", [E, NS], F8, kind="ExternalInput")
    wt_d = nc.dram_tensor("wt", [C, wt_cols], F16, kind="ExternalInput")
    b2_d = (nc.dram_tensor("b2m", [E, C], F16, kind="ExternalInput")
            if hasb2 else None)
    out_d = nc.dram_tensor("out", [C, NS], F8, kind="ExternalOutput")

    with tile.TileContext(nc) as tc:
        with (
            tc.tile_pool(name="consts", bufs=1) as consts,
            tc.tile_pool(name="xp", bufs=1) as xp,
            tc.tile_pool(name="fpool", bufs=3) as fpool,
            tc.tile_pool(name="gpool", bufs=3) as gpool,
            tc.tile_pool(name="opool", bufs=2) as opool,
        ):
            # ---------- loads ----------
            x_sb = xp.tile([C, NS], F8)
            wcm_sb = consts.tile([E, NS], F8)
            w1T = consts.tile([C, E * C], F16)
            w2T = consts.tile([C, E * C], F16)
            sel32 = consts.tile([E, E * C], F32)
            sel = consts.tile([E, E * C], F16)

            dmas = []
            for j in range(4):
                s = slice(j * (NS // 4), (j + 1) * (NS // 4))
                dmas.append(nc.sync.dma_start(x_sb[:, s], x8_d[:, s]))
            dmas.append(nc.sync.dma_start(wcm_sb[:], wc_d[:]))
            dmas.append(nc.sync.dma_start(w1T[:], wt_d[:, 0:E * C]))
            dmas.append(nc.sync.dma_start(w2T[:], wt_d[:, E * C:2 * E * C]))
            if hasb1:
                b1m = consts.tile([C, E], F32)
                bh = consts.tile([C, E], F16)
                dmas.append(nc.sync.dma_start(
                    bh[:], wt_d[:, 2 * E * C:2 * E * C + E]))
                nc.scalar.copy(b1m[:], bh[:])
            if hasb2:
                b2m = consts.tile([E, C], F16)
                dmas.append(nc.sync.dma_start(b2m[:], b2_d[:]))

            # on-device constant: expert-broadcast selector
            # sel[e, e*C:(e+1)*C] = 1, else 0
            nc.vector.memset(sel32[:], 1.0)
            nc.gpsimd.affine_select(
                sel32[:], sel32[:], pattern=[[1, E * C]], base=0,
                channel_multiplier=-C,
                compare_op=mybir.AluOpType.is_ge, fill=0.0)
            nc.gpsimd.affine_select(
                sel32[:], sel32[:], pattern=[[-1, E * C]], base=C - 1,
                channel_multiplier=C,
                compare_op=mybir.AluOpType.is_ge, fill=0.0)
            sel_cp = nc.scalar.copy(sel[:], sel32[:])

            # PE carries only ONE sync wait per Matmult through walrus;
            # absorb each input-DMA dependency into a PE nop up front.
            dma_nops = []
            for dma in dmas:
                nop = nc.tensor.nop(nofuse=True)
                add_dep_helper(nop.ins, dma.ins, sync=True)
                dma_nops.append(nop)

            def pe_absorb(producers, consumer_mms):
                nops = []
                for p in producers:
                    if p is None:
                        continue
                    n = nc.tensor.nop(nofuse=True)
                    add_dep_helper(n.ins, p.ins, sync=True)
                    nops.append(n)
                for m in consumer_mms:
                    for n in nops:
                        add_dep_helper(m.ins, n.ins, sync=False)

            # ---------- experts + combine ----------
            mpsum = tc.tile_pool(name="ps_m", bufs=1, space="PSUM")
            ps_m = mpsum.__enter__()
            mpsum2 = tc.tile_pool(name="ps_m2", bufs=2, space="PSUM")
            ps_m2 = mpsum2.__enter__()
            prev_out = prev_l2last = None
            hist_silu = [None, None]
            hist_mult = [None, None]
            hist_hmm = [None, None]
            hist_wb = [None, None]
            for i in range(NCHUNKS):
                cs = slice(i * NC_CHUNK, (i + 1) * NC_CHUNK)
                pso = ps_m.tile([C, NC_CHUNK], F32, tag="pso")
                for e in range(E):
                    mms = []
                    psh = ps_m2.tile([C, NC_CHUNK], F32, tag="psh")
                    for s in range(NC_CHUNK // 512):
                        rs = slice(i * NC_CHUNK + s * 512,
                                   i * NC_CHUNK + (s + 1) * 512)
                        mms.append(nc.tensor.matmul(
                            psh[:, s * 512:(s + 1) * 512],
                            w1T[:, e * C:(e + 1) * C],
                            x_sb[:, rs],
                            start=True, stop=True))
                    f_sb = fpool.tile([C, NC_CHUNK], F16, tag="f")
                    if hasb1:
                        silu_ins = nc.scalar.activation(
                            f_sb[:], psh[:], act_fn, bias=b1m[:, e:e + 1])
                    else:
                        silu_ins = nc.scalar.activation(f_sb[:], psh[:], act_fn)
                    pswb = ps_m.tile([C, NC_CHUNK], F32, tag="pswb")
                    for s in range(NC_CHUNK // 512):
                        ws = slice(i * NC_CHUNK + s * 512,
                                   i * NC_CHUNK + (s + 1) * 512)
                        mms.append(nc.tensor.matmul(
                            pswb[:, s * 512:(s + 1) * 512],
                            sel[:, e * C:(e + 1) * C], wcm_sb[:, ws],
                            start=True, stop=True))
                    g_sb = gpool.tile([C, NC_CHUNK], F16, tag="g")
                    mult_ins = nc.vector.tensor_mul(g_sb[:], f_sb[:], pswb[:])
                    for s in range(NC_CHUNK // 512):
                        ss = slice(s * 512, (s + 1) * 512)
                        mms.append(nc.tensor.matmul(
                            pso[:, ss],
                            w2T[:, e * C:(e + 1) * C],
                            g_sb[:, ss],
                            start=(e == 0),
                            stop=(e == E - 1) and not hasb2))
                    # absorb all cross-engine + psum-WAW deps into PE nops
                    pe_absorb([hist_silu[0], hist_mult[-1], prev_out,
                               hist_hmm[0], hist_wb[-1], prev_l2last,
                               sel_cp if (i == 0 and e == 0) else None],
                              mms[:1])
                    for m in mms[1:]:
                        add_dep_helper(m.ins, mms[0].ins, sync=False)
                    pe_absorb([silu_ins, mult_ins], mms[-NC_CHUNK // 512:])
                    hist_silu = [hist_silu[-1], silu_ins]
                    hist_mult = [hist_mult[-1], mult_ins]
                    hist_hmm = [hist_hmm[-1], mms[1]]
                    hist_wb = [hist_wb[-1], mms[NC_CHUNK // 512 + 1]]
                    if e == E - 1:
                        prev_l2last = mms[-1]
                if hasb2:
                    for s in range(NC_CHUNK // 512):
                        ss = slice(s * 512, (s + 1) * 512)
                        rs = slice(i * NC_CHUNK + s * 512,
                                   i * NC_CHUNK + (s + 1) * 512)
                        nc.tensor.matmul(
                            pso[:, ss], b2m[:], wcm_sb[:, rs],
                            start=False, stop=True)
                # delta only — the +x residual happens on host in f32
                o_sb = opool.tile([C, NC_CHUNK], F8, tag="o")
                prev_out = nc.scalar.copy(o_sb[:], pso[:])
                nc.sync.dma_start(out_d[:, cs], o_sb[:])
            mpsum2.__exit__(None, None, None)
            mpsum.__exit__(None, None, None)
    _split_waits(nc)
    return nc


# --- host side -------------------------------------------------------------
# Replicates bass_utils.run_bass_kernel_spmd's axon execution path
# (shard_map over jax.devices()[:8] + the _bass_exec_p custom call), but
# builds the jitted callable ONCE and reuses it: re-tracing and re-lowering
# on every call costs seconds of host time while computing nothing new.
_state: dict = {}


def _get_state(key):
    if key in _state:
        return _state[key]
    install_neuronx_cc_hook()
    nc = build_kernel(*key)
    partition_name = (nc.partition_id_tensor.name
                      if nc.partition_id_tensor else None)
    in_names, out_names, out_avals = [], [], []
    for alloc in nc.m.functions[0].allocations:
        if not isinstance(alloc, mybir.MemoryLocationSet):
            continue
        name = alloc.memorylocations[0].name
        if alloc.kind == "ExternalInput":
            if name != partition_name:
                in_names.append(name)
        elif alloc.kind == "ExternalOutput":
            out_names.append(name)
            out_avals.append(jax.core.ShapedArray(
                tuple(alloc.tensor_shape), mybir.dt.np(alloc.dtype)))
    n_params = len(in_names)
    n_outs = len(out_avals)
    all_names = tuple(in_names + out_names
                      + ([partition_name] if partition_name else []))

    def _body(*args):
        operands = list(args)
        if partition_name is not None:
            operands.append(partition_id_tensor())
        outs = _bass_exec_p.bind(
            *operands,
            out_avals=tuple(out_avals),
            in_names=all_names,
            out_names=tuple(out_names),
            lowering_input_output_aliases=(),
            sim_require_finite=True,
            sim_require_nnan=True,
            nc=nc,
        )
        return tuple(outs)

    devices = jax.devices()[:NCORES]
    mesh = Mesh(np.asarray(devices), ("core",))
    sharding = NamedSharding(mesh, PartitionSpec("core"))
    donate = tuple(range(n_params, n_params + n_outs))
    njit = jax.jit(
        shard_map(_body, mesh=mesh,
                  in_specs=(PartitionSpec("core"),) * (n_params + n_outs),
                  out_specs=(PartitionSpec("core"),) * n_outs,
                  check_rep=False),
        donate_argnums=donate, keep_unused=True)
    mkdonor = jax.jit(
        lambda: jnp.zeros((NCORES * C, NS), NP_F8), out_shardings=sharding)
    st = {"nc": nc, "njit": njit, "mkdonor": mkdonor, "sharding": sharding,
          "devices": devices, "in_names": in_names, "donors": [],
          "wt_sig": None, "wt_dev": None, "compiled": None}
    _state[key] = st
    return st


def kernel(x, gate_w, gate_b, w1, b1, w2, b2):
    x = np.asarray(x, dtype=np.float32)
    gate_w = np.asarray(gate_w, dtype=np.float32)
    gate_b = np.asarray(gate_b, dtype=np.float32)
    w1 = np.asarray(w1, dtype=np.float32)
    b1 = np.asarray(b1, dtype=np.float32)
    w2 = np.asarray(w2, dtype=np.float32)
    b2 = np.asarray(b2, dtype=np.float32)

    key = (bool(b1.any()), bool(b2.any()))
    st = _get_state(key)

    xr = x.reshape(B, C, THW)

    # x split shards, quantized to fp8, shipped as one async device_put
    # (a single put returns ~40ms sooner than 8 per-device puts on this
    # 1-CPU host, and the upload wire time is identical). The transfer is
    # in flight during the host gating compute below.
    x5 = x.reshape(B, C, CPB, NSPLIT, NS)

    def put_x(sp):
        # one strided gather+quantize pass: [B,CPB,C,NS] row-blocks land
        # exactly at core c = bb*CPB + q
        xg = _to_f8(x5[:, :, :, sp, :].transpose(0, 2, 1, 3)
                    ).reshape(NCORES * C, NS)
        return jax.device_put(xg, st["sharding"])

    x8_dev0 = put_x(0)   # upload in flight during the host work below

    # weights: device-resident across calls; re-uploaded only on change
    cached = st["wt_sig"]
    if (cached is None or not np.array_equal(cached[0], w1)
            or not np.array_equal(cached[1], w2)
            or not np.array_equal(cached[2], b1)):
        wt_cols = 2 * E * C + (E if key[0] else 0)
        wpk = np.empty((C, wt_cols), dtype=np.float16)
        wpk[:, 0:E * C] = w1.T
        wpk[:, E * C:2 * E * C] = w2.transpose(2, 0, 1).reshape(C, E * C)
        if key[0]:
            wpk[:, 2 * E * C:] = b1.reshape(E, C).T
        st["wt_dev"] = jax.device_put(np.tile(wpk, (NCORES, 1)),
                                      st["sharding"])
        st["wt_sig"] = (w1.copy(), w2.copy(), b1.copy())

    # --- gating on host, exact f32, one split at a time ---
    xr4 = x.reshape(B, C, CPB, NSH)
    has_gb = bool(gate_b.any())

    def gate_split(sp):
        lo = sp * NS
        # per-core strided-view gemms: 5x faster than tensordot (which
        # copies the slice before BLAS)
        lg = np.empty((E, NCORES, NS), dtype=np.float32)
        for c in range(NCORES):
            bb, q = divmod(c, CPB)
            o = q * NSH + lo
            np.matmul(gate_w, xr[bb, :, o:o + NS], out=lg[:, c, :])
        if has_gb:
            lg += gate_b[:, None, None]
        lt = lg.reshape(E, -1)
        n2 = lt.shape[1]
        ar = _AR_N2
        i1 = np.argmax(lt, axis=0)
        v1 = lt[i1, ar]
        lt[i1, ar] = -np.inf
        i2 = np.argmax(lt, axis=0)
        v2 = lt[i2, ar]
        ez = 1.0 / (1.0 + np.exp(v2 - v1))             # top-2 softmax
        w32 = np.zeros((E, n2), dtype=np.float32)
        w32[i1, ar] = ez
        w32[i2, ar] = 1.0 - ez
        return _to_f8(
            w32.reshape(E, NCORES, NS).transpose(1, 0, 2)
        ).reshape(NCORES * E, NS)

    b2t = (np.tile(b2.astype(np.float16), (NCORES, 1)) if key[1] else None)
    donors = st["donors"]
    st["donors"] = []

    def dispatch(sp, x8_dev):
        args_np = {"x8": x8_dev, "wc": gate_split(sp), "wt": st["wt_dev"],
                   "b2m": b2t}
        args = [args_np[n] for n in st["in_names"]]
        donor = donors.pop() if donors else st["mkdonor"]()
        fn = st["compiled"] or st["njit"]
        return fn(*args, donor)[0]

    # Issue order is the device queue order: dispatch split sp and kick
    # its result-fetch asyncs (pre-queued, so the data streams back the
    # moment exec sp finishes) BEFORE uploading split sp+1 — the fetch of
    # sp overlaps the upload of sp+1 on the full-duplex link.
    outs = [None] * NSPLIT
    for sp in range(NSPLIT):
        outs[sp] = dispatch(sp, x8_dev0 if sp == 0 else put_x(sp))
        for s in outs[sp].addressable_shards:
            try:
                s.data.copy_to_host_async()
            except Exception:
                pass
    # drain: per split, one gather of the (already async-copied) shards
    # and one 256-entry-LUT pass f8 -> f32, then 8 strided residual adds.
    # Batching per split halves the python/dispatch overhead of 32 tiny
    # per-shard conversions.
    out = np.empty((B, C, THW), dtype=np.float32)
    for sp in range(NSPLIT):
        d32 = _LUT_F8_TO_F32[np.asarray(outs[sp]).view(np.uint8)]
        for c in range(NCORES):
            bb, q = divmod(c, CPB)
            lo = q * NSH + sp * NS
            np.add(xr[bb, :, lo:lo + NS], d32[c * C:(c + 1) * C],
                   out=out[bb, :, lo:lo + NS])
        st["donors"].append(outs[sp])   # recycle as future donations
    return out.reshape(B, C, T, H, W)


# revision 42
# speedup vs baseline: 1.1272x; 1.0288x over previous
import sys

sys.path.insert(0, "/opt/trn_rl_repo")

import numpy as np

import jax
import jax.numpy as jnp
from jax.sharding import Mesh, NamedSharding, PartitionSpec
from jax.experimental.shard_map import shard_map

import concourse.bass as bass
import concourse.tile as tile
from concourse import mybir
from concourse.tile import add_dep_helper
from concourse.bass2jax import (
    _bass_exec_p, install_neuronx_cc_hook, partition_id_tensor)

# Problem constants (nn_MoEBlock: B,C,T,H,W = 2,128,8,64,64; E=8; top-2)
B, C, T, H, W = 2, 128, 8, 64, 64
E = 8
THW = T * H * W               # 32768
NVOX = B * THW                # 65536 voxels
NCORES = 8
NSH = NVOX // NCORES          # 8192 voxels per core
CPB = NCORES // B             # cores per batch element (4)
NSPLIT = 4                    # pipeline depth: device exec and both wire
                              # directions overlap across splits (the
                              # per-device queue is FIFO in issue order,
                              # but up- and down-transfers are full duplex)
NS = NSH // NSPLIT            # voxels per core per call
NC_CHUNK = 1024               # main-loop chunk (voxels)
NCHUNKS = NS // NC_CHUNK
F16 = mybir.dt.float16
F32 = mybir.dt.float32
F8 = mybir.dt.float8e4
NP_F8 = mybir.dt.np(F8)       # ml_dtypes.float8_e4m3

# ml_dtypes' elementwise f8 casts go through a slow scalar ufunc path on
# this 1-CPU host; table lookups are ~2x faster for both directions.
with np.errstate(invalid="ignore", over="ignore"):
    # f32 -> f16 (SIMD) -> f8 via 64K-entry table (double rounding is
    # harmless at our 13x error margin)
    _LUT_F16_TO_F8 = (np.arange(65536, dtype=np.uint16).view(np.float16)
                      .astype(NP_F8))
    # f8 -> f32 via 256-entry table
    _LUT_F8_TO_F32 = (np.arange(256, dtype=np.uint8).view(NP_F8)
                      .astype(np.float32))


def _to_f8(a32):
    """fast f32 ndarray -> f8e4m3 ndarray (any shape/strides)"""
    return _LUT_F16_TO_F8[a32.astype(np.float16).view(np.uint16)]


_AR_N2 = np.arange(NCORES * NS)   # voxel index vector reused per split


def _split_waits(nc, max_waits=1):
    """The walrus scheduler accepts only one sync-wait per instruction.
    Move extra on_wait conditions onto standalone same-engine NoOps
    inserted immediately before the instruction (same engine stream =>
    identical semantics)."""
    ctr = 0
    for f in nc.m.functions:
        for bb in f.blocks:
            insts = list(bb.instructions)
            out = []
            changed = False
            for inst in insts:
                si = inst.sync_info
                w = list(si.on_wait) if si is not None and si.on_wait else []
                if (len(w) > max_waits
                        and inst.engine != mybir.EngineType.Unassigned):
                    for extra in w[:-max_waits]:
                        ctr += 1
                        nop = mybir.InstNoOp(
                            name=f"WSPLIT-{ctr}", ins=[], outs=[])
                        nop.engine = inst.engine
                        nop.sync_info = mybir.SyncInfo(
                            on_wait=[extra], on_update=[])
                        out.append(nop)
                    inst.sync_info = mybir.SyncInfo(
                        on_wait=w[-max_waits:],
                        on_update=list(si.on_update) if si.on_update else [])
                    changed = True
                out.append(inst)
            if changed:
                try:
                    bb.instructions = out
                except Exception:
                    bb.instructions.clear()
                    bb.instructions.extend(out)
    return nc


def build_kernel(hasb1: bool, hasb2: bool):
    """Expert MLP only — gating (logits/top-2/softmax) happens on host in
    exact f32; the per-voxel expert weights arrive as the f8 `wc` input.
    Output is the MoE delta (no +x residual; host adds it in f32)."""
    act_fn = mybir.ActivationFunctionType.Silu
    wt_cols = 2 * E * C + (E if hasb1 else 0)
    nc = bass.Bass()
    x8_d = nc.dram_tensor("x8", [C, NS], F8, kind="ExternalInput")
    wc_d = nc.dram_tensor("wc", [E, NS], F8, kind="ExternalInput")
    wt_d = nc.dram_tensor("wt", [C, wt_cols], F16, kind="ExternalInput")
    b2_d = (nc.dram_tensor("b2m", [E, C], F16, kind="ExternalInput")
            if hasb2 else None)
    out_d = nc.dram_tensor("out", [C, NS], F8, kind="ExternalOutput")

    with tile.TileContext(nc) as tc:
        with (
            tc.tile_pool(name="consts", bufs=1) as consts,
            tc.tile_pool(name="xp", bufs=1) as xp,
            tc.tile_pool(name="fpool", bufs=3) as fpool,
            tc.tile_pool(name="gpool", bufs=3) as gpool,
            tc.tile_pool(name="opool", bufs=2) as opool,
        ):
            # ---------- loads ----------
            x_sb = xp.tile([C, NS], F8)
            wcm_sb = consts.tile([E, NS], F8)
            w1T = consts.tile([C, E * C], F16)
            w2T = consts.tile([C, E * C], F16)
            sel32 = consts.tile([E, E * C], F32)
            sel = consts.tile([E, E * C], F16)

            dmas = []
            for j in range(4):
                s = slice(j * (NS // 4), (j + 1) * (NS // 4))
                dmas.append(nc.sync.dma_start(x_sb[:, s], x8_d[:, s]))
            dmas.append(nc.sync.dma_start(wcm_sb[:], wc_d[:]))
            dmas.append(nc.sync.dma_start(w1T[:], wt_d[:, 0:E * C]))
            dmas.append(nc.sync.dma_start(w2T[:], wt_d[:, E * C:2 * E * C]))
            if hasb1:
                b1m = consts.tile([C, E], F32)
                bh = consts.tile([C, E], F16)
                dmas.append(nc.sync.dma_start(
                    bh[:], wt_d[:, 2 * E * C:2 * E * C + E]))
                nc.scalar.copy(b1m[:], bh[:])
            if hasb2:
                b2m = consts.tile([E, C], F16)
                dmas.append(nc.sync.dma_start(b2m[:], b2_d[:]))

            # on-device constant: expert-broadcast selector
            # sel[e, e*C:(e+1)*C] = 1, else 0
            nc.vector.memset(sel32[:], 1.0)
            nc.gpsimd.affine_select(
                sel32[:], sel32[:], pattern=[[1, E * C]], base=0,
                channel_multiplier=-C,
                compare_op=mybir.AluOpType.is_ge, fill=0.0)
            nc.gpsimd.affine_select(
                sel32[:], sel32[:], pattern=[[-1, E * C]], base=C - 1,
                channel_multiplier=C,
                compare_op=mybir.AluOpType.is_ge, fill=0.0)
            sel_cp = nc.scalar.copy(sel[:], sel32[:])

            # PE carries only ONE sync wait per Matmult through walrus;
            # absorb each input-DMA dependency into a PE nop up front.
            dma_nops = []
            for dma in dmas:
                nop = nc.tensor.nop(nofuse=True)
                add_dep_helper(nop.ins, dma.ins, sync=True)
                dma_nops.append(nop)

            def pe_absorb(producers, consumer_mms):
                nops = []
                for p in producers:
                    if p is None:
                        continue
                    n = nc.tensor.nop(nofuse=True)
                    add_dep_helper(n.ins, p.ins, sync=True)
                    nops.append(n)
                for m in consumer_mms:
                    for n in nops:
                        add_dep_helper(m.ins, n.ins, sync=False)

            # ---------- experts + combine ----------
            mpsum = tc.tile_pool(name="ps_m", bufs=1, space="PSUM")
            ps_m = mpsum.__enter__()
            mpsum2 = tc.tile_pool(name="ps_m2", bufs=2, space="PSUM")
            ps_m2 = mpsum2.__enter__()
            prev_out = prev_l2last = None
            hist_silu = [None, None]
            hist_mult = [None, None]
            hist_hmm = [None, None]
            hist_wb = [None, None]
            for i in range(NCHUNKS):
                cs = slice(i * NC_CHUNK, (i + 1) * NC_CHUNK)
                pso = ps_m.tile([C, NC_CHUNK], F32, tag="pso")
                for e in range(E):
                    mms = []
                    psh = ps_m2.tile([C, NC_CHUNK], F32, tag="psh")
                    for s in range(NC_CHUNK // 512):
                        rs = slice(i * NC_CHUNK + s * 512,
                                   i * NC_CHUNK + (s + 1) * 512)
                        mms.append(nc.tensor.matmul(
                            psh[:, s * 512:(s + 1) * 512],
                            w1T[:, e * C:(e + 1) * C],
                            x_sb[:, rs],
                            start=True, stop=True))
                    f_sb = fpool.tile([C, NC_CHUNK], F16, tag="f")
                    if hasb1:
                        silu_ins = nc.scalar.activation(
                            f_sb[:], psh[:], act_fn, bias=b1m[:, e:e + 1])
                    else:
                        silu_ins = nc.scalar.activation(f_sb[:], psh[:], act_fn)
                    pswb = ps_m.tile([C, NC_CHUNK], F32, tag="pswb")
                    for s in range(NC_CHUNK // 512):
                        ws = slice(i * NC_CHUNK + s * 512,
                                   i * NC_CHUNK + (s + 1) * 512)
                        mms.append(nc.tensor.matmul(
                            pswb[:, s * 512:(s + 1) * 512],
                            sel[:, e * C:(e + 1) * C], wcm_sb[:, ws],
                            start=True, stop=True))
                    g_sb = gpool.tile([C, NC_CHUNK], F16, tag="g")
                    mult_ins = nc.vector.tensor_mul(g_sb[:], f_sb[:], pswb[:])
                    for s in range(NC_CHUNK // 512):
                        ss = slice(s * 512, (s + 1) * 512)
                        mms.append(nc.tensor.matmul(
                            pso[:, ss],
                            w2T[:, e * C:(e + 1) * C],
                            g_sb[:, ss],
                            start=(e == 0),
                            stop=(e == E - 1) and not hasb2))
                    # absorb all cross-engine + psum-WAW deps into PE nops
                    pe_absorb([hist_silu[0], hist_mult[-1], prev_out,
                               hist_hmm[0], hist_wb[-1], prev_l2last,
                               sel_cp if (i == 0 and e == 0) else None],
                              mms[:1])
                    for m in mms[1:]:
                        add_dep_helper(m.ins, mms[0].ins, sync=False)
                    pe_absorb([silu_ins, mult_ins], mms[-NC_CHUNK // 512:])
                    hist_silu = [hist_silu[-1], silu_ins]
                    hist_mult = [hist_mult[-1], mult_ins]
                    hist_hmm = [hist_hmm[-1], mms[1]]
                    hist_wb = [hist_wb[-1], mms[NC_CHUNK // 512 + 1]]
                    if e == E - 1:
                        prev_l2last = mms[-1]
                if hasb2:
                    for s in range(NC_CHUNK // 512):
                        ss = slice(s * 512, (s + 1) * 512)
                        rs = slice(i * NC_CHUNK + s * 512,
                                   i * NC_CHUNK + (s + 1) * 512)
                        nc.tensor.matmul(
                            pso[:, ss], b2m[:], wcm_sb[:, rs],
                            start=False, stop=True)
                # delta only — the +x residual happens on host in f32
                o_sb = opool.tile([C, NC_CHUNK], F8, tag="o")
                prev_out = nc.scalar.copy(o_sb[:], pso[:])
                nc.sync.dma_start(out_d[:, cs], o_sb[:])
            mpsum2.__exit__(None, None, None)
            mpsum.__exit__(None, None, None)
    _split_waits(nc)
    return nc


# --- host side -------------------------------------------------------------
# Replicates bass_utils.run_bass_kernel_spmd's axon execution path
# (shard_map over jax.devices()[:8] + the _bass_exec_p custom call), but
# builds the jitted callable ONCE and reuses it: re-tracing and re-lowering
# on every call costs seconds of host time while computing nothing new.
_state: dict = {}


def _get_state(key):
    if key in _state:
        return _state[key]
    install_neuronx_cc_hook()
    nc = build_kernel(*key)
    partition_name = (nc.partition_id_tensor.name
                      if nc.partition_id_tensor else None)
    in_names, out_names, out_avals = [], [], []
    for alloc in nc.m.functions[0].allocations:
        if not isinstance(alloc, mybir.MemoryLocationSet):
            continue
        name = alloc.memorylocations[0].name
        if alloc.kind == "ExternalInput":
            if name != partition_name:
                in_names.append(name)
        elif alloc.kind == "ExternalOutput":
            out_names.append(name)
            out_avals.append(jax.core.ShapedArray(
                tuple(alloc.tensor_shape), mybir.dt.np(alloc.dtype)))
    n_params = len(in_names)
    n_outs = len(out_avals)
    all_names = tuple(in_names + out_names
                      + ([partition_name] if partition_name else []))

    def _body(*args):
        operands = list(args)
        if partition_name is not None:
            operands.append(partition_id_tensor())
        outs = _bass_exec_p.bind(
            *operands,
            out_avals=tuple(out_avals),
            in_names=all_names,
            out_names=tuple(out_names),
            lowering_input_output_aliases=(),
            sim_require_finite=True,
            sim_require_nnan=True,
            nc=nc,
        )
        return tuple(outs)

    devices = jax.devices()[:NCORES]
    mesh = Mesh(np.asarray(devices), ("core",))
    sharding = NamedSharding(mesh, PartitionSpec("core"))
    donate = tuple(range(n_params, n_params + n_outs))
    njit = jax.jit(
        shard_map(_body, mesh=mesh,
                  in_specs=(PartitionSpec("core"),) * (n_params + n_outs),
                  out_specs=(PartitionSpec("core"),) * n_outs,
                  check_rep=False),
        donate_argnums=donate, keep_unused=True)
    mkdonor = jax.jit(
        lambda: jnp.zeros((NCORES * C, NS), NP_F8), out_shardings=sharding)
    st = {"nc": nc, "njit": njit, "mkdonor": mkdonor, "sharding": sharding,
          "devices": devices, "in_names": in_names, "donors": [],
          "wt_sig": None, "wt_dev": None, "compiled": None}
    _state[key] = st
    return st


def kernel(x, gate_w, gate_b, w1, b1, w2, b2):
    x = np.asarray(x, dtype=np.float32)
    gate_w = np.asarray(gate_w, dtype=np.float32)
    gate_b = np.asarray(gate_b, dtype=np.float32)
    w1 = np.asarray(w1, dtype=np.float32)
    b1 = np.asarray(b1, dtype=np.float32)
    w2 = np.asarray(w2, dtype=np.float32)
    b2 = np.asarray(b2, dtype=np.float32)

    key = (bool(b1.any()), bool(b2.any()))
    st = _get_state(key)

    xr = x.reshape(B, C, THW)

    # x split shards, quantized to fp8, shipped as one async device_put
    # (a single put returns ~40ms sooner than 8 per-device puts on this
    # 1-CPU host, and the upload wire time is identical). The transfer is
    # in flight during the host gating compute below.
    x5 = x.reshape(B, C, CPB, NSPLIT, NS)

    def put_x(sp):
        # one strided gather+quantize pass: [B,CPB,C,NS] row-blocks land
        # exactly at core c = bb*CPB + q
        xg = _to_f8(x5[:, :, :, sp, :].transpose(0, 2, 1, 3)
                    ).reshape(NCORES * C, NS)
        return jax.device_put(xg, st["sharding"])

    x8_dev0 = put_x(0)   # upload in flight during the host work below

    # weights: device-resident across calls; re-uploaded only on change
    cached = st["wt_sig"]
    if (cached is None or not np.array_equal(cached[0], w1)
            or not np.array_equal(cached[1], w2)
            or not np.array_equal(cached[2], b1)):
        wt_cols = 2 * E * C + (E if key[0] else 0)
        wpk = np.empty((C, wt_cols), dtype=np.float16)
        wpk[:, 0:E * C] = w1.T
        wpk[:, E * C:2 * E * C] = w2.transpose(2, 0, 1).reshape(C, E * C)
        if key[0]:
            wpk[:, 2 * E * C:] = b1.reshape(E, C).T
        st["wt_dev"] = jax.device_put(np.tile(wpk, (NCORES, 1)),
                                      st["sharding"])
        st["wt_sig"] = (w1.copy(), w2.copy(), b1.copy())

    # --- gating on host, exact f32, one split at a time ---
    xr4 = x.reshape(B, C, CPB, NSH)
    has_gb = bool(gate_b.any())

    def gate_split(sp):
        lo = sp * NS
        # per-core strided-view gemms: 5x faster than tensordot (which
        # copies the slice before BLAS)
        lg = np.empty((E, NCORES, NS), dtype=np.float32)
        for c in range(NCORES):
            bb, q = divmod(c, CPB)
            o = q * NSH + lo
            np.matmul(gate_w, xr[bb, :, o:o + NS], out=lg[:, c, :])
        if has_gb:
            lg += gate_b[:, None, None]
        lt = lg.reshape(E, -1)
        n2 = lt.shape[1]
        ar = _AR_N2
        i1 = np.argmax(lt, axis=0)
        v1 = lt[i1, ar]
        lt[i1, ar] = -np.inf
        i2 = np.argmax(lt, axis=0)
        v2 = lt[i2, ar]
        ez = 1.0 / (1.0 + np.exp(v2 - v1))             # top-2 softmax
        # build the f8 bytes directly: only 2 of E entries per voxel are
        # nonzero, so convert just the two ez vectors and scatter bytes —
        # skips an 8.4MB f32 memset + a full-array f8 conversion pass
        wq = np.zeros((E, n2), dtype=np.uint8)
        wq[i1, ar] = _to_f8(ez).view(np.uint8)
        wq[i2, ar] = _to_f8(1.0 - ez).view(np.uint8)
        return np.ascontiguousarray(
            wq.reshape(E, NCORES, NS).transpose(1, 0, 2)
        ).view(NP_F8).reshape(NCORES * E, NS)

    b2t = (np.tile(b2.astype(np.float16), (NCORES, 1)) if key[1] else None)
    donors = st["donors"]
    st["donors"] = []

    def dispatch(sp, x8_dev):
        args_np = {"x8": x8_dev, "wc": gate_split(sp), "wt": st["wt_dev"],
                   "b2m": b2t}
        args = [args_np[n] for n in st["in_names"]]
        donor = donors.pop() if donors else st["mkdonor"]()
        fn = st["compiled"] or st["njit"]
        return fn(*args, donor)[0]

    # Issue order is the device queue order: dispatch split sp and kick
    # its result-fetch asyncs (pre-queued, so the data streams back the
    # moment exec sp finishes) BEFORE uploading split sp+1 — the fetch of
    # sp overlaps the upload of sp+1 on the full-duplex link.
    outs = [None] * NSPLIT
    for sp in range(NSPLIT):
        outs[sp] = dispatch(sp, x8_dev0 if sp == 0 else put_x(sp))
        for s in outs[sp].addressable_shards:
            try:
                s.data.copy_to_host_async()
            except Exception:
                pass
    # drain: per split, one gather of the (already async-copied) shards
    # and one 256-entry-LUT pass f8 -> f32, then 8 strided residual adds.
    # Batching per split halves the python/dispatch overhead of 32 tiny
    # per-shard conversions.
    out = np.empty((B, C, THW), dtype=np.float32)
    for sp in range(NSPLIT):
        d32 = _LUT_F8_TO_F32[np.asarray(outs[sp]).view(np.uint8)]
        for c in range(NCORES):
            bb, q = divmod(c, CPB)
            lo = q * NSH + sp * NS
            np.add(xr[bb, :, lo:lo + NS], d32[c * C:(c + 1) * C],
                   out=out[bb, :, lo:lo + NS])
        st["donors"].append(outs[sp])   # recycle as future donations
    return out.reshape(B, C, T, H, W)
